# revision 1
# baseline (speedup 1.0000x reference)
"""Single-head attention (B=8, S=4096, E=512, H=64) on 8 trn2 NeuronCores.

Sharding: data-parallel over batch — one batch element per core.

Per-core algorithm (batch b):
  - Host pre-transposes x[b] -> xT [E, S] (layout prep only).
  - QKV: Q^T,K^T [H, S] head-major and V' [S, H+1] S-major (ones column
    appended) via PE matmuls over E-chunks; biases folded in as K=1
    rank-1 matmuls (bias x ones-row).
  - Scores computed TRANSPOSED: S^T[sk, sq] = K^T.T @ Q^T so softmax runs
    along partitions and attn @ V needs no transpose of attn.
  - Mask applied additively PRE-exp using the PE's free lhsT transpose:
    S^T += mask_chunk.T @ (-32768 * I); the int32 mask is DMA'd with an
    SWDGE dtype-cast straight to bf16 {0,1} (exact), so masking costs no
    vector-engine passes at all. exp(scale*(qk - 32768*m)) underflows to
    exactly 0 on masked lanes.
  - exp on ACT with no max-subtraction (|scaled scores| < ~10, safe).
  - Softmax denominator comes free from the ones column of V':
    outT = V'.T @ attn^T accumulates [H+1, sq] where row H is the row sum.
  - Tiny fixup: PE-transpose outT, reciprocal + scale on the [128, 64]
    output, DMA out.

Everything fp32 except: mask path (bf16, exact for {0,1} / -32768) and,
in "f32r" precision mode, the two big matmul groups (scores, attn@V) plus
QKV operand storage, which use float32r (~1.5e-4 matmul rel err, 4x faster
than fp32 on the PE).
"""
import sys

sys.path.insert(0, "/opt/trn_rl_repo")

import numpy as np

import concourse.bacc as bacc
import concourse.tile as tile
from concourse import mybir
from concourse.bass_utils import run_bass_kernel_spmd

F32 = mybir.dt.float32
F32R = mybir.dt.float32r
BF16 = mybir.dt.bfloat16
I32 = mybir.dt.int32

B, S, E, H = 8, 4096, 512, 64
SCALE = float(E) ** -0.5
NEG = -32768.0

PREC = "f32r"  # "f32" (exact) or "f32r" (fast PE mode for big matmuls)
MASK_CAST_DMA = False  # SWDGE int32->bf16 cast during DMA; else DVE convert


def build_program(s=S, prec=PREC, mask_cast=MASK_CAST_DMA):
    nc = bacc.Bacc("TRN2", target_bir_lowering=False, debug=False, num_devices=B)
    xT = nc.dram_tensor("xT", [E, s], F32, kind="ExternalInput")
    mask = nc.dram_tensor("mask", [s, s], I32, kind="ExternalInput")
    wq = nc.dram_tensor("wq", [E, H], F32, kind="ExternalInput")
    wk = nc.dram_tensor("wk", [E, H], F32, kind="ExternalInput")
    wv = nc.dram_tensor("wv", [E, H], F32, kind="ExternalInput")
    bq = nc.dram_tensor("bq", [1, H], F32, kind="ExternalInput")
    bk = nc.dram_tensor("bk", [1, H], F32, kind="ExternalInput")
    bv1 = nc.dram_tensor("bv1", [1, H + 1], F32, kind="ExternalInput")
    out = nc.dram_tensor("out", [s, H], F32, kind="ExternalOutput")

    DT = F32R if prec == "f32r" else F32
    NE = E // 128          # 4 E-chunks
    NB = s // 512          # q/s blocks of 512
    NQ = s // 128          # 128-row chunks

    with tile.TileContext(nc) as tc:
        with (
            tc.tile_pool(name="const", bufs=1) as cst,
            tc.tile_pool(name="xp", bufs=2) as xp,
            tc.tile_pool(name="qkv", bufs=1) as qkv,
            tc.tile_pool(name="maskp", bufs=7) as maskp,
            tc.tile_pool(name="maskip", bufs=6) as maskip,
            tc.tile_pool(name="atp", bufs=3) as atp,
            tc.tile_pool(name="osb", bufs=2) as osb,
        ):
            # ---- constants ----
            negI = cst.tile([128, 128], BF16)
            nc.gpsimd.memset(negI, 0.0)
            nc.gpsimd.affine_select(
                out=negI, in_=negI, compare_op=mybir.AluOpType.not_equal,
                fill=NEG, base=0, pattern=[[-1, 128]], channel_multiplier=1,
            )
            idf = cst.tile([128, 128], F32)
            nc.gpsimd.memset(idf, 0.0)
            nc.gpsimd.affine_select(
                out=idf, in_=idf, compare_op=mybir.AluOpType.not_equal,
                fill=1.0, base=0, pattern=[[-1, 128]], channel_multiplier=1,
            )
            ones512 = cst.tile([1, 512], F32)
            nc.vector.memset(ones512, 1.0)
            ones128 = cst.tile([1, 128], F32)
            nc.vector.memset(ones128, 1.0)

            wq_sb = cst.tile([128, NE, H], F32)
            wk_sb = cst.tile([128, NE, H], F32)
            wv_sb = cst.tile([128, NE, H], F32)
            wq_r = cst.tile([128, NE, H], DT)
            wk_r = cst.tile([128, NE, H], DT)
            wv_r = cst.tile([128, NE, H], DT)
            for w_dram, w_sb, w_r in (
                (wq, wq_sb, wq_r), (wk, wk_sb, wk_r), (wv, wv_sb, wv_r)
            ):
                nc.sync.dma_start(
                    out=w_sb, in_=w_dram.rearrange("(c p) h -> p c h", p=128)
                )
                nc.vector.tensor_copy(w_r, w_sb)
            bv1_sb = cst.tile([1, H + 1], F32)
            nc.sync.dma_start(out=bv1_sb, in_=bv1[:])
            bqt = cst.tile([H, 1], F32)
            bkt = cst.tile([H, 1], F32)
            nc.sync.dma_start(out=bqt, in_=bq[:].rearrange("a h -> h a"))
            nc.sync.dma_start(out=bkt, in_=bk[:].rearrange("a h -> h a"))

            # ---- mask DMA (issue first: no deps, fills DMA queues early) ----
            mbs = []
            for qb in range(NB):
                row = []
                for j in range(4):
                    q0 = qb * 512 + j * 128
                    mb = maskp.tile([128, s], BF16, tag="mb", name=f"mb_{qb}_{j}")
                    if mask_cast:
                        nc.gpsimd.dma_start(out=mb, in_=mask[q0:q0 + 128, :])
                    else:
                        hw = max(s // 2, 512)
                        for hh in range(0, s, hw):
                            mi = maskip.tile(
                                [128, hw], I32, tag="mi", name=f"mi_{qb}_{j}_{hh}"
                            )
                            nc.sync.dma_start(
                                out=mi, in_=mask[q0:q0 + 128, hh:hh + hw]
                            )
                            nc.vector.tensor_copy(mb[:, hh:hh + hw], mi)
                    row.append(mb)
                mbs.append(row)

            # ---- phase A: QT, KT head-major; V' S-major (per-block tiles) ----
            QTb = [qkv.tile([H, 512], DT, name=f"qt_{i}") for i in range(NB)]
            KTb = [qkv.tile([H, 512], DT, name=f"kt_{i}") for i in range(NB)]
            VPk = [qkv.tile([128, H + 1], DT, name=f"vp_{i}") for i in range(NQ)]
            with tc.tile_pool(name="psA", bufs=2, space="PSUM") as psA:
                for sb in range(NB):
                    s0 = sb * 512
                    xt = xp.tile([128, NE, 512], F32, tag="xt", name=f"xt_{sb}", bufs=1)
                    xtr = xp.tile([128, NE, 512], DT, tag="xtr", name=f"xtr_{sb}")
                    half = NE // 2
                    for eh in range(2):
                        e0 = eh * half
                        nc.scalar.dma_start(
                            out=xt[:, e0:e0 + half, :],
                            in_=xT[e0 * 128:(e0 + half) * 128, s0:s0 + 512]
                            .rearrange("(c p) s -> p c s", p=128),
                        )
                        nc.vector.tensor_copy(
                            xtr[:, e0:e0 + half, :], xt[:, e0:e0 + half, :]
                        )
                    # Q and K matmuls interleaved -> alternate PSUM banks
                    q_ps = psA.tile([H, 512], F32, tag="qk", name=f"q_ps_{sb}")
                    k_ps = psA.tile([H, 512], F32, tag="qk", name=f"k_ps_{sb}")
                    for e in range(NE):
                        nc.tensor.matmul(q_ps, wq_r[:, e, :], xtr[:, e, :],
                                         start=(e == 0), stop=(e == NE - 1))
                        nc.tensor.matmul(k_ps, wk_r[:, e, :], xtr[:, e, :],
                                         start=(e == 0), stop=(e == NE - 1))
                    nc.scalar.activation(QTb[sb], q_ps,
                                         mybir.ActivationFunctionType.Identity,
                                         bias=bqt)
                    nc.scalar.activation(KTb[sb], k_ps,
                                         mybir.ActivationFunctionType.Identity,
                                         bias=bkt)
                    # V chunks in bank-alternating pairs
                    for j0 in range(0, 4, 2):
                        vps = [
                            psA.tile([128, H + 1], F32, tag="v",
                                     name=f"v_ps_{sb}_{j0 + jj}")
                            for jj in range(2)
                        ]
                        for jj in range(2):
                            nc.tensor.matmul(vps[jj], ones128, bv1_sb,
                                             start=True, stop=False)
                        for e in range(NE):
                            for jj in range(2):
                                c0 = (j0 + jj) * 128
                                nc.tensor.matmul(
                                    vps[jj][:, 0:H], xtr[:, e, c0:c0 + 128],
                                    wv_r[:, e, :], start=False, stop=(e == NE - 1),
                                )
                        for jj in range(2):
                            nc.vector.tensor_copy(VPk[sb * 4 + j0 + jj], vps[jj])

            # ---- phase B: scores^T (+mask), exp, attn@V, fixup ----
            with (
                tc.tile_pool(name="psS", bufs=2, space="PSUM") as psS,
                tc.tile_pool(name="psO", bufs=2, space="PSUM") as psO,
                tc.tile_pool(name="psF", bufs=2, space="PSUM") as psF,
            ):
                for qb in range(NB):
                    q0 = qb * 512
                    ot_ps = psO.tile([H + 1, 512], F32, tag="ot")
                    for g in range(NQ // 2):
                        sc = psS.tile([128, 1024], F32, tag="sc")
                        for h2 in range(2):
                            k = 2 * g + h2
                            nc.tensor.matmul(
                                sc[:, 512 * h2:512 * h2 + 512],
                                KTb[k // 4][:, 128 * (k % 4):128 * (k % 4 + 1)],
                                QTb[qb],
                                start=True, stop=False,
                            )
                        for j in range(4):
                            for h2 in range(2):
                                k = 2 * g + h2
                                c = 512 * h2 + 128 * j
                                nc.tensor.matmul(
                                    sc[:, c:c + 128],
                                    mbs[qb][j][:, 128 * k:128 * (k + 1)],
                                    negI, start=False, stop=(j == 3),
                                )
                        at = atp.tile([128, 1024], DT, tag="at")
                        nc.scalar.activation(
                            at, sc, mybir.ActivationFunctionType.Exp, scale=SCALE
                        )
                        for h2 in range(2):
                            k = 2 * g + h2
                            nc.tensor.matmul(
                                ot_ps, VPk[k], at[:, 512 * h2:512 * h2 + 512],
                                start=(k == 0), stop=(k == NQ - 1),
                            )
                    oT = osb.tile([H + 1, 512], F32, tag="oT")
                    nc.vector.tensor_copy(oT, ot_ps)
                    for j in range(4):
                        fx = psF.tile([128, H + 1], F32, tag="fx")
                        nc.tensor.transpose(
                            fx, oT[:, 128 * j:128 * (j + 1)], idf[0:H + 1, 0:H + 1]
                        )
                        ob = osb.tile([128, H + 1], F32, tag="ob")
                        nc.vector.tensor_copy(ob, fx)
                        rc = osb.tile([128, 1], F32, tag="rc")
                        nc.vector.reciprocal(rc, ob[:, H:H + 1])
                        of = osb.tile([128, H], F32, tag="of")
                        nc.vector.tensor_scalar_mul(of, ob[:, 0:H], rc)
                        nc.gpsimd.dma_start(
                            out=out[q0 + 128 * j:q0 + 128 * (j + 1), :], in_=of
                        )
    nc.compile()
    return nc


def make_in_maps(x, attention_mask, Wq, bq, Wk, bk, Wv, bv):
    nb = x.shape[0]
    bv1 = np.concatenate([bv, np.ones(1, np.float32)]).reshape(1, H + 1)
    common = {
        "wq": np.ascontiguousarray(Wq), "wk": np.ascontiguousarray(Wk),
        "wv": np.ascontiguousarray(Wv),
        "bq": np.ascontiguousarray(bq.reshape(1, H)),
        "bk": np.ascontiguousarray(bk.reshape(1, H)),
        "bv1": bv1,
    }
    return [
        {
            "xT": np.ascontiguousarray(x[b].T),
            "mask": np.ascontiguousarray(attention_mask[b]),
            **common,
        }
        for b in range(nb)
    ]


_PROGRAM = None


def kernel(x, attention_mask, Wq, bq, Wk, bk, Wv, bv):
    global _PROGRAM
    x = np.asarray(x, np.float32)
    attention_mask = np.asarray(attention_mask, np.int32)
    if _PROGRAM is None:
        _PROGRAM = build_program()
    in_maps = make_in_maps(
        x, attention_mask,
        np.asarray(Wq, np.float32), np.asarray(bq, np.float32),
        np.asarray(Wk, np.float32), np.asarray(bk, np.float32),
        np.asarray(Wv, np.float32), np.asarray(bv, np.float32),
    )
    res = run_bass_kernel_spmd(_PROGRAM, in_maps, core_ids=list(range(B)))
    return np.stack([res.results[b]["out"] for b in range(B)], axis=0)



# revision 5
# speedup vs baseline: 1.5303x; 1.5303x over previous
"""Single-head attention (B=8, S=4096, E=512, H=64) on 8 trn2 NeuronCores.

Sharding: data-parallel over batch - one batch element per core.

v2 design (PE-light, ACT(exp)-roof ~128us):
  - Host marshaling: x[b].T cast to bf16; mask transposed+inverted to
    m01T = (mask.T == 0) in bf16 {0,1}; weights bf16 with Wq/Wk duplicated
    column-wise ([Wq|Wq]) so the Q/K projections produce a vertically
    duplicated [128, S] layout for free (same matmul stream, M=128).
  - Scores computed TRANSPOSED and ROW-TILED: chunk pair (k1,k2) runs as two
    concurrent K=64 matmuls on row-groups 0-63 / 64-127 (tile_position),
    halving PE score time.
  - Mask applied two ways (split tunable, balances PE vs DVE):
      PE pairs:  sc += 32768 * m01T via stationary posI (= diag(+32768)
                 stacked twice), streamed m01T as rhs; exp bias -32768*scale
                 restores unmasked scores and underflows masked ones to 0.
      DVE pairs: at = exp(scale*sc) then at *= m01T (bf16 tensor_tensor 2x).
  - Softmax denominator free via ones-column of V' (M=65 attn@V).
  - exp on ACT from PSUM at FD=1024 (one instruction per chunk pair).
"""
import sys

sys.path.insert(0, "/opt/trn_rl_repo")

import ml_dtypes
import numpy as np

import concourse.bacc as bacc
import concourse.tile as tile
from concourse import mybir
from concourse.bass_utils import run_bass_kernel_spmd

F32 = mybir.dt.float32
BF16 = mybir.dt.bfloat16
NPBF16 = ml_dtypes.bfloat16

B, S, E, H = 8, 4096, 512, 64
SCALE = float(E) ** -0.5
POS = 32768.0
EXP_BIAS = -float(np.float32(POS) * np.float32(SCALE))

NPE_PAIRS = 4  # pairs per q-block masked on PE (rest: DVE multiply)


def build_program(s=S, npe=NPE_PAIRS):
    nc = bacc.Bacc("TRN2", target_bir_lowering=False, debug=False, num_devices=B)
    xT = nc.dram_tensor("xT", [E, s], BF16, kind="ExternalInput")
    m01T = nc.dram_tensor("m01T", [s, s], BF16, kind="ExternalInput")
    wq2 = nc.dram_tensor("wq2", [E, 128], BF16, kind="ExternalInput")
    wk2 = nc.dram_tensor("wk2", [E, 128], BF16, kind="ExternalInput")
    wv = nc.dram_tensor("wv", [E, H], BF16, kind="ExternalInput")
    b2q = nc.dram_tensor("b2q", [128, 1], F32, kind="ExternalInput")
    b2k = nc.dram_tensor("b2k", [128, 1], F32, kind="ExternalInput")
    bv1 = nc.dram_tensor("bv1", [1, H + 1], F32, kind="ExternalInput")
    out = nc.dram_tensor("out", [s, H], F32, kind="ExternalOutput")

    NE = E // 128          # 4 E-chunks
    NB = s // 512          # 8 blocks of 512 (both s and q blocking)
    NQ = s // 128          # 32 key chunks of 128

    with tile.TileContext(nc) as tc:
        with (
            tc.tile_pool(name="const", bufs=1) as cst,
            tc.tile_pool(name="xp", bufs=3) as xp,
            tc.tile_pool(name="qkv", bufs=1) as qkv,
            tc.tile_pool(name="mstr", bufs=3) as mstr,
            tc.tile_pool(name="atp", bufs=3) as atp,
            tc.tile_pool(name="osb", bufs=2) as osb,
            tc.tile_pool(name="psS", bufs=2, space="PSUM") as psS,
            tc.tile_pool(name="psQK", bufs=2, space="PSUM") as psQK,
            tc.tile_pool(name="psO", bufs=2, space="PSUM") as psO,
        ):
            # ---- constants ----
            posI = cst.tile([128, 64], BF16)
            nc.gpsimd.memset(posI, 0.0)
            nc.gpsimd.affine_select(
                out=posI, in_=posI, compare_op=mybir.AluOpType.not_equal,
                fill=POS, base=0, pattern=[[-1, 64]], channel_multiplier=1,
            )
            nc.gpsimd.affine_select(
                out=posI, in_=posI, compare_op=mybir.AluOpType.not_equal,
                fill=POS, base=-64, pattern=[[-1, 64]], channel_multiplier=1,
            )
            idf = cst.tile([128, 128], F32)
            nc.gpsimd.memset(idf, 0.0)
            nc.gpsimd.affine_select(
                out=idf, in_=idf, compare_op=mybir.AluOpType.not_equal,
                fill=1.0, base=0, pattern=[[-1, 128]], channel_multiplier=1,
            )
            ones1 = cst.tile([1, 128], F32)
            nc.vector.memset(ones1, 1.0)
            ebias = cst.tile([128, 1], F32)
            nc.vector.memset(ebias, EXP_BIAS)
            zbias = cst.tile([128, 1], F32)
            nc.vector.memset(zbias, 0.0)

            wq2_sb = cst.tile([128, NE, 128], BF16)
            wk2_sb = cst.tile([128, NE, 128], BF16)
            wv_sb = cst.tile([128, NE, H], BF16)
            nc.sync.dma_start(
                out=wq2_sb, in_=wq2.rearrange("(c p) m -> p c m", p=128))
            nc.sync.dma_start(
                out=wk2_sb, in_=wk2.rearrange("(c p) m -> p c m", p=128))
            nc.sync.dma_start(
                out=wv_sb, in_=wv.rearrange("(c p) h -> p c h", p=128))
            bv1_sb = cst.tile([1, H + 1], F32)
            nc.sync.dma_start(out=bv1_sb, in_=bv1[:])
            b2q_sb = cst.tile([128, 1], F32)
            b2k_sb = cst.tile([128, 1], F32)
            nc.sync.dma_start(out=b2q_sb, in_=b2q[:])
            nc.sync.dma_start(out=b2k_sb, in_=b2k[:])

            # ---- mask stripe DMA (one 4MiB transfer per q-block) ----
            stripes = []
            for qb in range(NB):
                st = mstr.tile([128, NQ, 512], BF16, tag="st", name=f"st_{qb}")
                nc.sync.dma_start(
                    out=st,
                    in_=m01T[:, qb * 512:(qb + 1) * 512]
                    .rearrange("(c p) q -> p c q", p=128),
                )
                stripes.append(st)

            # ---- phase A: Q2/K2 (dup-stacked, bf16) and V' per 512-block ----
            Q2 = [qkv.tile([128, 512], BF16, name=f"q2_{i}") for i in range(NB)]
            K2 = [qkv.tile([128, 512], BF16, name=f"k2_{i}") for i in range(NB)]
            VP = [qkv.tile([128, H + 1], BF16, name=f"vp_{i}") for i in range(NQ)]
            for sb in range(NB):
                s0 = sb * 512
                xt = xp.tile([128, NE, 512], BF16, tag="xt", name=f"xt_{sb}")
                nc.scalar.dma_start(
                    out=xt,
                    in_=xT[:, s0:s0 + 512].rearrange("(c p) s -> p c s", p=128),
                )
                q_ps = psQK.tile([128, 512], F32, tag="qk", name=f"q_ps_{sb}")
                k_ps = psQK.tile([128, 512], F32, tag="qk", name=f"k_ps_{sb}")
                for e in range(NE):
                    nc.tensor.matmul(q_ps, wq2_sb[:, e, :], xt[:, e, :],
                                     start=(e == 0), stop=(e == NE - 1))
                    nc.tensor.matmul(k_ps, wk2_sb[:, e, :], xt[:, e, :],
                                     start=(e == 0), stop=(e == NE - 1))
                nc.vector.tensor_scalar_add(Q2[sb], q_ps, b2q_sb)
                nc.vector.tensor_scalar_add(K2[sb], k_ps, b2k_sb)
                for j in range(4):
                    c0 = j * 128
                    v_ps = psO.tile([128, H + 1], F32, tag="o",
                                    name=f"v_ps_{sb}_{j}")
                    nc.tensor.matmul(v_ps, ones1, bv1_sb, start=True, stop=False)
                    for e in range(NE):
                        nc.tensor.matmul(
                            v_ps[:, 0:H], xt[:, e, c0:c0 + 128], wv_sb[:, e, :],
                            start=False, stop=(e == NE - 1),
                        )
                    nc.vector.tensor_copy(VP[sb * 4 + j], v_ps)

            # ---- phase B: rowtiled scores, mask, exp, attn@V, fixup ----
            for qb in range(NB):
                q0 = qb * 512
                st = stripes[qb]
                ot_ps = psO.tile([H + 1, 512], F32, tag="o", name=f"ot_{qb}")
                for g in range(NQ // 2):
                    k1, k2 = 2 * g, 2 * g + 1
                    pe_mask = g < npe
                    sc = psS.tile([128, 1024], F32, tag="sc")
                    nc.tensor.matmul(
                        sc[:, 0:512],
                        K2[k1 // 4][0:64, (k1 % 4) * 128:(k1 % 4 + 1) * 128],
                        Q2[qb][0:64, :],
                        start=True, stop=not pe_mask, tile_position=(0, 0),
                    )
                    nc.tensor.matmul(
                        sc[:, 512:1024],
                        K2[k2 // 4][64:128, (k2 % 4) * 128:(k2 % 4 + 1) * 128],
                        Q2[qb][64:128, :],
                        start=True, stop=not pe_mask, tile_position=(64, 0),
                    )
                    if pe_mask:
                        for h2, k in ((0, k1), (1, k2)):
                            c = 512 * h2
                            nc.tensor.matmul(
                                sc[0:64, c:c + 512], posI[0:64, :],
                                st[0:64, k, :],
                                start=False, stop=True, tile_position=(0, 0),
                            )
                            nc.tensor.matmul(
                                sc[64:128, c:c + 512], posI[64:128, :],
                                st[64:128, k, :],
                                start=False, stop=True, tile_position=(64, 64),
                            )
                    at = atp.tile([128, 1024], BF16, tag="at")
                    nc.scalar.activation(
                        at, sc, mybir.ActivationFunctionType.Exp,
                        scale=SCALE, bias=ebias if pe_mask else zbias,
                    )
                    if not pe_mask:
                        nc.vector.tensor_mul(at, at, st[:, k1:k1 + 2, :])
                    nc.tensor.matmul(ot_ps, VP[k1], at[:, 0:512],
                                     start=(g == 0), stop=False)
                    nc.tensor.matmul(ot_ps, VP[k2], at[:, 512:1024],
                                     start=False, stop=(g == NQ // 2 - 1))
                oT = osb.tile([H + 1, 512], F32, tag="oT")
                nc.vector.tensor_copy(oT, ot_ps)
                of = osb.tile([128, 4, H], F32, tag="of")
                for j in range(4):
                    fx = psQK.tile([128, H + 1], F32, tag="qk", name=f"fx_{qb}_{j}")
                    nc.tensor.transpose(
                        fx, oT[:, 128 * j:128 * (j + 1)], idf[0:H + 1, 0:H + 1]
                    )
                    rc = osb.tile([128, 1], F32, tag="rc")
                    nc.vector.reciprocal(rc, fx[:, H:H + 1])
                    nc.vector.tensor_scalar_mul(of[:, j, :], fx[:, 0:H], rc)
                nc.gpsimd.dma_start(
                    out=out[q0:q0 + 512, :].rearrange("(j p) h -> p j h", p=128),
                    in_=of,
                )
    nc.compile()
    return nc


def make_in_maps(x, attention_mask, Wq, bq, Wk, bk, Wv, bv):
    nb = x.shape[0]
    wq2 = np.concatenate([Wq, Wq], axis=1).astype(NPBF16)
    wk2 = np.concatenate([Wk, Wk], axis=1).astype(NPBF16)
    b2q = np.concatenate([bq, bq]).reshape(128, 1).astype(np.float32)
    b2k = np.concatenate([bk, bk]).reshape(128, 1).astype(np.float32)
    bv1 = np.concatenate([bv, np.ones(1, np.float32)]).reshape(1, H + 1)
    common = {
        "wq2": wq2, "wk2": wk2, "wv": np.asarray(Wv).astype(NPBF16),
        "b2q": b2q, "b2k": b2k, "bv1": bv1.astype(np.float32),
    }
    return [
        {
            "xT": np.ascontiguousarray(x[b].T).astype(NPBF16),
            "m01T": (attention_mask[b].T == 0).astype(NPBF16),
            **common,
        }
        for b in range(nb)
    ]


_PROGRAM = None


def kernel(x, attention_mask, Wq, bq, Wk, bk, Wv, bv):
    global _PROGRAM
    x = np.asarray(x, np.float32)
    attention_mask = np.asarray(attention_mask, np.int32)
    if _PROGRAM is None:
        _PROGRAM = build_program()
    in_maps = make_in_maps(
        x, attention_mask,
        np.asarray(Wq, np.float32), np.asarray(bq, np.float32),
        np.asarray(Wk, np.float32), np.asarray(bk, np.float32),
        np.asarray(Wv, np.float32), np.asarray(bv, np.float32),
    )
    res = run_bass_kernel_spmd(_PROGRAM, in_maps, core_ids=list(range(B)))
    return np.stack([res.results[b]["out"] for b in range(B)], axis=0)
